# revision 1
# baseline (speedup 1.0000x reference)
"""Trainium2 Bass kernel for causal self-attention (B=4, T=2048, C=2048, H=16).

Sharding: 8 cores = DP4 (batch) x TP2 (8 heads each). Each core:
  P1  qk' = (x @ Wqk)^T computed directly in [j, t] layout (j = head-dim rows)
  P2  v   = x @ Wv in natural [t, j] layout (stationary = xT tiles)
  P3  per head: s_T = k'^T q' -> exp -> causal mask -> av + replicated-ones den
      -> y'_h = o' * recip(den)    (y' kept in [c', t] layout for proj)
  P4  out_partial[t, n] = y'^T @ Wp (stationary = y' tiles) + bias (even core)
  P5  ReduceScatter(add) over core pairs -> each core owns half the t rows.

All matmuls bf16 with fp32 PSUM accumulation; softmax in fp32 on ACT/DVE.
Host side: shard/cast/transpose inputs, assemble output.
"""
import os
import math
import numpy as np
import ml_dtypes

import concourse.bass as bass
import concourse.bacc as bacc
import concourse.mybir as mybir
import concourse.tile as tile

F32 = mybir.dt.float32
BF16 = mybir.dt.bfloat16
AF = mybir.ActivationFunctionType

D = 128          # head dim (fixed: partition size)
N_CORES = 8
PAIRS = [[0, 1], [2, 3], [4, 5], [6, 7]]


class Cfg:
    def __init__(self, T=2048, H_TOT=16, HPC=8, B=4):
        self.T = T                    # sequence length
        self.H_TOT = H_TOT            # total heads
        self.HPC = HPC                # heads per core
        self.B = B
        self.C = H_TOT * D            # model dim
        self.CP = HPC * D             # per-core head cols
        self.TCH = 512                # ti chunk width
        assert T % self.TCH == 0 and T % D == 0


def build_kernel(cfg: Cfg):
    T, C, CP, HPC, TCH = cfg.T, cfg.C, cfg.CP, cfg.HPC, cfg.TCH
    NC_CH = C // D                # c-chunks (contraction)
    NJB = 2 * HPC                 # qk' j-blocks (q heads then k heads)
    NTB = T // D                  # t-blocks
    NIC = T // TCH                # ti chunks
    NTR = T // 512                # t-ranges for P1 moving dim
    NPR = C // 512                # proj n-ranges
    scale = 1.0 / math.sqrt(D)

    nc = bacc.Bacc()
    xT = nc.declare_dram_parameter("xT", [C, T], BF16, isOutput=False)
    wqk = nc.declare_dram_parameter("wqk", [C, 2 * CP], BF16, isOutput=False)
    wv = nc.declare_dram_parameter("wv", [C, CP], BF16, isOutput=False)
    wp = nc.declare_dram_parameter("wp", [CP, C], BF16, isOutput=False)
    bqk = nc.declare_dram_parameter("bqk", [D, NJB], F32, isOutput=False)
    bv = nc.declare_dram_parameter("bv", [1, CP], BF16, isOutput=False)
    bp = nc.declare_dram_parameter("bp", [1, C], BF16, isOutput=False)
    masks = nc.declare_dram_parameter("masks", [D, 4 * TCH], BF16, isOutput=False)
    out_ext = nc.declare_dram_parameter("out", [T // 2, C], F32, isOutput=True)

    qk_dram = nc.dram_tensor("qk_dram", [HPC, 2, D, T], BF16)
    partial_dram = nc.dram_tensor("partial_dram", [T, C], F32)
    rs_out = nc.dram_tensor("rs_out", [T // 2, C], F32)

    with tile.TileContext(nc) as tc:
        with (
            tc.tile_pool(name="const", bufs=1) as constp,
            tc.tile_pool(name="vres", bufs=1) as vres,
            tc.tile_pool(name="yres", bufs=1) as yres,
        ):
            bqk_t = constp.tile([D, NJB], F32, name="bqk_t")
            nc.sync.dma_start(bqk_t[:], bqk[:, :])
            bv_t = constp.tile([1, CP], BF16, name="bv_t")
            nc.sync.dma_start(bv_t[:], bv[:, :])
            bp_t = constp.tile([1, C], BF16, name="bp_t")
            nc.sync.dma_start(bp_t[:], bp[:, :])
            # masks: partition dim must be D -> load as [D, 4*TCH]
            mask_sb = constp.tile([D, 4 * TCH], BF16, name="mask_sb")
            nc.sync.dma_start(mask_sb[:], masks[:, :])
            ones_sq = constp.tile([D, D], BF16, name="ones_sq")
            nc.vector.memset(ones_sq[:], 1.0)
            ones_row = constp.tile([1, D], BF16, name="ones_row")
            nc.vector.memset(ones_row[:], 1.0)

            # resident xT tiles (bf16), one per c-chunk; freed after P2
            xtp_ctx = tc.tile_pool(name="xtp", bufs=1)
            xtp = xtp_ctx.__enter__()
            xt = []
            for c in range(NC_CH):
                t = xtp.tile([D, T], BF16, name=f"xt{c}")
                nc.sync.dma_start(t[:], xT[c * D:(c + 1) * D, :])
                xt.append(t)

            # ---------------- P1: qk' ----------------
            with (
                tc.tile_pool(name="wqkp", bufs=2 * NC_CH) as wqkp,
                tc.tile_pool(name="pq", bufs=8, space="PSUM") as pqp,
                tc.tile_pool(name="qkst", bufs=4) as qkst,
            ):
                for jb in range(NJB):
                    wts = []
                    for c in range(NC_CH):
                        wt = wqkp.tile([D, D], BF16, name="wt", tag="wt")
                        nc.sync.dma_start(
                            wt[:], wqk[c * D:(c + 1) * D, jb * D:(jb + 1) * D])
                        wts.append(wt)
                    ps = [pqp.tile([D, 512], F32, name="pq", tag="pq")
                          for _ in range(NTR)]
                    for c in range(NC_CH):
                        for tr in range(NTR):
                            nc.tensor.matmul(
                                ps[tr][:], wts[c][:],
                                xt[c][:, tr * 512:(tr + 1) * 512],
                                start=(c == 0), stop=(c == NC_CH - 1))
                    st = qkst.tile([D, T], BF16, name="qks", tag="qks")
                    for tr in range(NTR):
                        nc.vector.tensor_scalar_add(
                            st[:, tr * 512:(tr + 1) * 512], ps[tr][:],
                            bqk_t[:, jb:jb + 1])
                    nc.sync.dma_start(qk_dram[jb // 2, jb % 2, :, :], st[:])

            # ---------------- P2: v ----------------
            v_sb = []
            with (
                tc.tile_pool(name="wvp", bufs=1) as wvp,
                tc.tile_pool(name="pv", bufs=4, space="PSUM") as pvp,
            ):
                wv_t = []
                for c in range(NC_CH):
                    t = wvp.tile([D, CP], BF16, name=f"wv{c}")
                    nc.sync.dma_start(t[:], wv[c * D:(c + 1) * D, :])
                    wv_t.append(t)
                NVR = CP // 512 if CP >= 512 else 1
                VRW = min(512, CP)
                for tb in range(NTB):
                    ps = [pvp.tile([D, VRW], F32, name="pv", tag="pv")
                          for _ in range(NVR)]
                    for c in range(NC_CH):
                        for vr in range(NVR):
                            nc.tensor.matmul(
                                ps[vr][:], xt[c][:, tb * D:(tb + 1) * D],
                                wv_t[c][:, vr * VRW:(vr + 1) * VRW],
                                start=(c == 0), stop=False)
                    for vr in range(NVR):
                        nc.tensor.matmul(
                            ps[vr][:], ones_row[:],
                            bv_t[:, vr * VRW:(vr + 1) * VRW],
                            start=False, stop=True)
                    vt = vres.tile([D, CP], BF16, name=f"v{tb}")
                    for vr in range(NVR):
                        nc.vector.tensor_copy(
                            vt[:, vr * VRW:(vr + 1) * VRW], ps[vr][:])
                    v_sb.append(vt)
            xtp_ctx.__exit__(None, None, None)

            # ---------------- P3: attention ----------------
            y_sb = []
            with (
                tc.tile_pool(name="qkio", bufs=2) as qkio,
                tc.tile_pool(name="attp", bufs=6) as attp,
                tc.tile_pool(name="ps_s", bufs=3, space="PSUM") as ps_s,
                tc.tile_pool(name="ps_o", bufs=2, space="PSUM") as ps_o,
                tc.tile_pool(name="ps_d", bufs=2, space="PSUM") as ps_d,
                tc.tile_pool(name="normp", bufs=4) as normp,
            ):
                for h in range(HPC):
                    qk_sb = qkio.tile([D, 2 * T], BF16, name="qk_sb", tag="qkio")
                    qp = qk_sb[:, 0:T]
                    kp = qk_sb[:, T:2 * T]
                    nc.sync.dma_start(qp, qk_dram[h, 0, :, :])
                    nc.sync.dma_start(kp, qk_dram[h, 1, :, :])
                    yt = yres.tile([D, T], BF16, name=f"y{h}")
                    for ic in range(NIC):
                        ti0 = ic * TCH
                        ntk = (ti0 + TCH) // D
                        po = ps_o.tile([D, TCH], F32, name="po", tag="po")
                        pd = ps_d.tile([D, TCH], F32, name="pd", tag="pd")
                        for tk in range(ntk):
                            s_p = ps_s.tile([D, TCH], F32, name="sp", tag="sp")
                            nc.tensor.matmul(
                                s_p[:], kp[:, tk * D:(tk + 1) * D],
                                qp[:, ti0:ti0 + TCH], start=True, stop=True)
                            kdiag = tk - (ntk - TCH // D)
                            if kdiag >= 0:
                                nc.vector.tensor_add(
                                    s_p[:], s_p[:],
                                    mask_sb[:, kdiag * TCH:(kdiag + 1) * TCH])
                            att = attp.tile([D, TCH], BF16, name="att", tag="att")
                            nc.scalar.activation(att[:], s_p[:], AF.Exp,
                                                 bias=0.0, scale=scale)
                            nc.tensor.matmul(
                                po[:], v_sb[tk][:, h * D:(h + 1) * D], att[:],
                                start=(tk == 0), stop=(tk == ntk - 1))
                            nc.tensor.matmul(
                                pd[:], ones_sq[:], att[:],
                                start=(tk == 0), stop=(tk == ntk - 1))
                        rec = normp.tile([D, TCH], F32, name="rec", tag="rec")
                        nc.vector.reciprocal(rec[:], pd[:])
                        nc.vector.tensor_mul(yt[:, ti0:ti0 + TCH], po[:], rec[:])
                    y_sb.append(yt)

            # ---------------- P4: proj partials ----------------
            with (
                tc.tile_pool(name="wpp", bufs=1) as wpp,
                tc.tile_pool(name="pp", bufs=8, space="PSUM") as ppp,
                tc.tile_pool(name="post", bufs=3) as post,
            ):
                wp_t = []
                for c in range(HPC):
                    t = wpp.tile([D, C], BF16, name=f"wp{c}")
                    nc.sync.dma_start(t[:], wp[c * D:(c + 1) * D, :])
                    wp_t.append(t)
                for tb in range(NTB):
                    ps = [ppp.tile([D, 512], F32, name="pp", tag="pp")
                          for _ in range(NPR)]
                    for c in range(HPC):
                        for nr in range(NPR):
                            nc.tensor.matmul(
                                ps[nr][:], y_sb[c][:, tb * D:(tb + 1) * D],
                                wp_t[c][:, nr * 512:(nr + 1) * 512],
                                start=(c == 0), stop=False)
                    for nr in range(NPR):
                        nc.tensor.matmul(
                            ps[nr][:], ones_row[:],
                            bp_t[:, nr * 512:(nr + 1) * 512],
                            start=False, stop=True)
                    st = post.tile([D, C], F32, name="pst", tag="pst")
                    for nr in range(NPR):
                        nc.vector.tensor_copy(
                            st[:, nr * 512:(nr + 1) * 512], ps[nr][:])
                    nc.sync.dma_start(partial_dram[tb * D:(tb + 1) * D, :], st[:])

            # ---------------- P5: ReduceScatter pairs ----------------
            nc.gpsimd.collective_compute(
                "ReduceScatter",
                mybir.AluOpType.add,
                ins=[partial_dram[:, :]],
                outs=[rs_out[:, :]],
                replica_groups=PAIRS,
            )
            nc.sync.dma_start(out_ext[:, :], rs_out[:, :])
    nc.finalize()
    return nc


def _prep_inputs(cfg: Cfg, x, w_attn, b_attn, w_proj, b_proj):
    """Host-side shard/cast. Returns in_maps (list of dicts per core)."""
    T, C, CP, HPC = cfg.T, cfg.C, cfg.CP, cfg.HPC
    bf = ml_dtypes.bfloat16
    wq = w_attn[:, 0:C]
    wk = w_attn[:, C:2 * C]
    wvf = w_attn[:, 2 * C:3 * C]
    bq, bk, bvf = b_attn[0:C], b_attn[C:2 * C], b_attn[2 * C:3 * C]

    masks = np.zeros((D, 4 * cfg.TCH), dtype=bf)
    f = np.arange(cfg.TCH)[None, :]
    p = np.arange(D)[:, None]
    for k in range(4):
        keep = (f - p >= 128 * k)
        masks[:, k * cfg.TCH:(k + 1) * cfg.TCH] = np.where(
            keep, 0.0, -30000.0).astype(bf)

    in_maps = []
    for core in range(N_CORES):
        b = core // 2
        g = core % 2
        h0 = g * HPC * D            # first col of this head group
        sl = slice(h0, h0 + CP)
        xTc = np.ascontiguousarray(x[b].T).astype(bf)
        wqk_cols = []
        for h in range(HPC):
            hs = slice(h0 + h * D, h0 + (h + 1) * D)
            wqk_cols.append(wq[:, hs])
            wqk_cols.append(wk[:, hs])
        wqk_c = np.concatenate(wqk_cols, axis=1).astype(bf)
        wv_c = wvf[:, sl].astype(bf)
        wp_c = w_proj[sl, :].astype(bf)
        bqk_cols = []
        for h in range(HPC):
            hs = slice(h0 + h * D, h0 + (h + 1) * D)
            bqk_cols.append(bq[hs])
            bqk_cols.append(bk[hs])
        bqk_c = np.ascontiguousarray(np.stack(bqk_cols, axis=1)).astype(np.float32)
        in_maps.append({
            "xT": xTc,
            "wqk": wqk_c,
            "wv": wv_c,
            "wp": wp_c,
            "bqk": bqk_c,
            "bv": bvf[sl].reshape(1, CP).astype(bf),
            "bp": (b_proj * (1.0 - g)).reshape(1, C).astype(bf),
            "masks": masks,
        })
    return in_maps


_CFG = Cfg()


def kernel(x, w_attn, b_attn, w_proj, b_proj, _trace=False, _cfg=None):
    from concourse.bass_utils import run_bass_kernel_spmd
    cfg = _cfg or _CFG
    x = np.asarray(x, dtype=np.float32)
    w_attn = np.asarray(w_attn, dtype=np.float32)
    b_attn = np.asarray(b_attn, dtype=np.float32)
    w_proj = np.asarray(w_proj, dtype=np.float32)
    b_proj = np.asarray(b_proj, dtype=np.float32)

    in_maps = _prep_inputs(cfg, x, w_attn, b_attn, w_proj, b_proj)
    nc = build_kernel(cfg)
    res = run_bass_kernel_spmd(nc, in_maps, list(range(N_CORES)), trace=_trace)
    outs = []
    for b in range(cfg.B):
        top = res.results[2 * b]["out"]
        bot = res.results[2 * b + 1]["out"]
        outs.append(np.concatenate([top, bot], axis=0))
    full = np.stack(outs, axis=0).astype(np.float32)
    if _trace:
        kernel.last_exec_time_ns = res.exec_time_ns
        kernel.last_mean_exec_time_ns = res.mean_exec_time_ns
        kernel.last_scope_times = res.per_core_scope_times
    return full



# revision 4
# speedup vs baseline: 1.2055x; 1.2055x over previous
"""Trainium2 Bass kernel for causal self-attention (B=4, T=2048, C=2048, H=16).

Sharding: 8 cores = DP4 (batch) x TP2 (8 heads each). Per core:
  A1  q,k = (x @ Wqk)^T in [j, t] layout; k kept resident in SBUF,
      q staged to DRAM in 512-col chunks.
  A2  v = x @ Wv in [t, j] layout (stationary xT blocks), resident in SBUF.
  B   per 512-query chunk ic, per head: s_T = k'^T q' (psum groups of 2
      key-blocks), causal mask add, exp ([128,1024] ACT), av + ones-den
      matmuls, rec = ACT reciprocal, y = po * rec.
      Then proj partials for the chunk (own 8 heads, bf16) and a chunked
      pair ReduceScatter(add, bf16) overlapped with the next chunk.
Host: shard/cast/transpose inputs, assemble output, upcast to fp32.
"""
import math
import numpy as np
import ml_dtypes

import concourse.bass as bass
import concourse.bacc as bacc
import concourse.mybir as mybir
import concourse.tile as tile

F32 = mybir.dt.float32
BF16 = mybir.dt.bfloat16
AF = mybir.ActivationFunctionType

D = 128          # head dim (fixed: partition size)
N_CORES = 8
PAIRS = [[0, 1], [2, 3], [4, 5], [6, 7]]


class Cfg:
    def __init__(self, T=2048, H_TOT=16, HPC=8, B=4):
        self.T = T                    # sequence length
        self.H_TOT = H_TOT            # total heads
        self.HPC = HPC                # heads per core
        self.B = B
        self.C = H_TOT * D            # model dim
        self.CP = HPC * D             # per-core head cols
        self.TCH = 512                # query chunk width
        assert T % self.TCH == 0 and T % D == 0


def build_kernel(cfg: Cfg):
    T, C, CP, HPC, TCH = cfg.T, cfg.C, cfg.CP, cfg.HPC, cfg.TCH
    NC_CH = C // D                # c-chunks (contraction)
    NTB = T // D                  # t-blocks
    NIC = T // TCH                # query chunks
    NTR = T // 512                # t-ranges for A1 moving dim
    NPR = C // TCH                # proj n-ranges
    HALF = TCH // 2               # rows owned per core per chunk
    scale = 1.0 / math.sqrt(D)

    nc = bacc.Bacc()
    xT = nc.declare_dram_parameter("xT", [C, T], BF16, isOutput=False)
    wqk = nc.declare_dram_parameter("wqk", [C, 2 * CP], BF16, isOutput=False)
    wv = nc.declare_dram_parameter("wv", [C, CP], BF16, isOutput=False)
    wp = nc.declare_dram_parameter("wp", [CP, C], BF16, isOutput=False)
    bqk = nc.declare_dram_parameter("bqk", [D, 2 * HPC], F32, isOutput=False)
    bv = nc.declare_dram_parameter("bv", [1, CP], BF16, isOutput=False)
    bp = nc.declare_dram_parameter("bp", [1, C], BF16, isOutput=False)
    masks = nc.declare_dram_parameter("masks", [D, 4 * TCH], BF16, isOutput=False)
    out_ext = nc.declare_dram_parameter("out", [NIC * HALF, C], BF16, isOutput=True)

    q_dram = nc.dram_tensor("q_dram", [HPC, NIC, D, TCH], BF16)
    part_dram = nc.dram_tensor("part_dram", [NIC, TCH, C], BF16)
    rs_out = nc.dram_tensor("rs_out", [NIC, HALF, C], BF16)

    with tile.TileContext(nc) as tc:
        with (
            tc.tile_pool(name="const", bufs=1) as constp,
            tc.tile_pool(name="kres", bufs=1) as kres,
            tc.tile_pool(name="vres", bufs=1) as vres,
        ):
            bqk_t = constp.tile([D, 2 * HPC], F32, name="bqk_t")
            nc.sync.dma_start(bqk_t[:], bqk[:, :])
            bv_t = constp.tile([1, CP], BF16, name="bv_t")
            nc.sync.dma_start(bv_t[:], bv[:, :])
            bp_t = constp.tile([1, C], BF16, name="bp_t")
            nc.sync.dma_start(bp_t[:], bp[:, :])
            mask_sb = constp.tile([D, 4 * TCH], BF16, name="mask_sb")
            nc.sync.dma_start(mask_sb[:], masks[:, :])
            ones_sq = constp.tile([D, D], BF16, name="ones_sq")
            nc.vector.memset(ones_sq[:], 1.0)
            ones_row = constp.tile([1, D], BF16, name="ones_row")
            nc.vector.memset(ones_row[:], 1.0)

            k_sb = [kres.tile([D, T], BF16, name=f"k{h}") for h in range(HPC)]
            v_sb = [vres.tile([D, CP], BF16, name=f"v{tb}") for tb in range(NTB)]

            # resident xT tiles, freed after A2
            xtp_ctx = tc.tile_pool(name="xtp", bufs=1)
            xtp = xtp_ctx.__enter__()
            xt = []
            for c in range(NC_CH):
                t = xtp.tile([D, T], BF16, name=f"xt{c}")
                nc.sync.dma_start(t[:], xT[c * D:(c + 1) * D, :])
                xt.append(t)

            # ---------------- A1: q (to DRAM) and k (resident) ----------
            with (
                tc.tile_pool(name="wqkp", bufs=2 * NC_CH) as wqkp,
                tc.tile_pool(name="pqk", bufs=8, space="PSUM") as pqkp,
                tc.tile_pool(name="qst", bufs=3) as qstp,
            ):
                for h in range(HPC):
                    for part in range(2):      # 0 = q, 1 = k
                        jb = 2 * h + part
                        wts = []
                        for c in range(NC_CH):
                            wt = wqkp.tile([D, D], BF16, name="wt", tag="wt")
                            nc.sync.dma_start(
                                wt[:], wqk[c * D:(c + 1) * D,
                                           jb * D:(jb + 1) * D])
                            wts.append(wt)
                        ps = [pqkp.tile([D, 512], F32, name="pq", tag="pq")
                              for _ in range(NTR)]
                        for c in range(NC_CH):
                            for tr in range(NTR):
                                nc.tensor.matmul(
                                    ps[tr][:], wts[c][:],
                                    xt[c][:, tr * 512:(tr + 1) * 512],
                                    start=(c == 0), stop=(c == NC_CH - 1))
                        if part == 1:
                            for tr in range(NTR):
                                nc.vector.tensor_scalar_add(
                                    k_sb[h][:, tr * 512:(tr + 1) * 512],
                                    ps[tr][:], bqk_t[:, jb:jb + 1])
                        else:
                            st = qstp.tile([D, T], BF16, name="qs", tag="qs")
                            for tr in range(NTR):
                                nc.vector.tensor_scalar_add(
                                    st[:, tr * 512:(tr + 1) * 512],
                                    ps[tr][:], bqk_t[:, jb:jb + 1])
                            for ic in range(NIC):
                                nc.sync.dma_start(
                                    q_dram[h, ic, :, :],
                                    st[:, ic * TCH:(ic + 1) * TCH])

            # ---------------- A2: v (resident) ---------------------------
            with (
                tc.tile_pool(name="wvp", bufs=1) as wvp,
                tc.tile_pool(name="pv", bufs=4, space="PSUM") as pvp,
            ):
                wv_t = []
                for c in range(NC_CH):
                    t = wvp.tile([D, CP], BF16, name=f"wv{c}")
                    nc.sync.dma_start(t[:], wv[c * D:(c + 1) * D, :])
                    wv_t.append(t)
                NVR = CP // 512
                for tb in range(NTB):
                    ps = [pvp.tile([D, 512], F32, name="pv", tag="pv")
                          for _ in range(NVR)]
                    for c in range(NC_CH):
                        for vr in range(NVR):
                            nc.tensor.matmul(
                                ps[vr][:], xt[c][:, tb * D:(tb + 1) * D],
                                wv_t[c][:, vr * 512:(vr + 1) * 512],
                                start=(c == 0), stop=False)
                    for vr in range(NVR):
                        nc.tensor.matmul(
                            ps[vr][:], ones_row[:],
                            bv_t[:, vr * 512:(vr + 1) * 512],
                            start=False, stop=True)
                    for vr in range(NVR):
                        nc.vector.tensor_copy(
                            v_sb[tb][:, vr * 512:(vr + 1) * 512], ps[vr][:])
            xtp_ctx.__exit__(None, None, None)

            # ---------------- B: attention + chunked proj + RS -----------
            with (
                tc.tile_pool(name="wpp", bufs=1) as wpp,
                tc.tile_pool(name="qio", bufs=16) as qio,
                tc.tile_pool(name="spool", bufs=2, space="PSUM") as spool,
                tc.tile_pool(name="acc", bufs=4, space="PSUM") as accp,
                tc.tile_pool(name="attp", bufs=3) as attp,
                tc.tile_pool(name="recp", bufs=2) as recp,
                tc.tile_pool(name="yres", bufs=12) as yresp,
                tc.tile_pool(name="pstg", bufs=3) as pstgp,
            ):
                wp_t = []
                for c in range(HPC):
                    t = wpp.tile([D, C], BF16, name=f"wp{c}")
                    nc.sync.dma_start(t[:], wp[c * D:(c + 1) * D, :])
                    wp_t.append(t)

                for ic in range(NIC):
                    ntk = 4 * (ic + 1)
                    qts = []
                    for h in range(HPC):
                        qt = qio.tile([D, TCH], BF16, name="qt", tag="qt")
                        nc.sync.dma_start(qt[:], q_dram[h, ic, :, :])
                        qts.append(qt)
                    ys = []
                    for h in range(HPC):
                        po = accp.tile([D, TCH], F32, name="po", tag="acc")
                        pd = accp.tile([D, TCH], F32, name="pd", tag="acc")
                        for g in range(ntk // 2):
                            s = spool.tile([D, 2 * TCH], F32, name="sp",
                                           tag="sp")
                            for j in range(2):
                                tk = 2 * g + j
                                nc.tensor.matmul(
                                    s[:, j * TCH:(j + 1) * TCH],
                                    k_sb[h][:, tk * D:(tk + 1) * D],
                                    qts[h][:], start=True, stop=True)
                            for j in range(2):
                                tk = 2 * g + j
                                kdiag = tk - (ntk - 4)
                                if kdiag >= 0:
                                    nc.vector.tensor_add(
                                        s[:, j * TCH:(j + 1) * TCH],
                                        s[:, j * TCH:(j + 1) * TCH],
                                        mask_sb[:, kdiag * TCH:
                                                (kdiag + 1) * TCH])
                            att = attp.tile([D, 2 * TCH], BF16, name="att",
                                            tag="att")
                            nc.scalar.activation(att[:], s[:], AF.Exp,
                                                 bias=0.0, scale=scale)
                            for j in range(2):
                                tk = 2 * g + j
                                nc.tensor.matmul(
                                    po[:], v_sb[tk][:, h * D:(h + 1) * D],
                                    att[:, j * TCH:(j + 1) * TCH],
                                    start=(tk == 0), stop=(tk == ntk - 1))
                            for j in range(2):
                                tk = 2 * g + j
                                nc.tensor.matmul(
                                    pd[:], ones_sq[:],
                                    att[:, j * TCH:(j + 1) * TCH],
                                    start=(tk == 0), stop=(tk == ntk - 1))
                        rec = recp.tile([D, TCH], F32, name="rec", tag="rec")
                        nc.vector.reciprocal_approx_fast(rec[:], pd[:])
                        yt = yresp.tile([D, TCH], BF16, name="yt", tag="yt")
                        nc.vector.tensor_mul(yt[:], po[:], rec[:])
                        ys.append(yt)

                    # proj partials for this chunk (own heads, bf16)
                    for tb in range(NIC):
                        pps = [accp.tile([D, TCH], F32, name="pp", tag="acc")
                               for _ in range(NPR)]
                        for c in range(HPC):
                            for nr in range(NPR):
                                nc.tensor.matmul(
                                    pps[nr][:],
                                    ys[c][:, tb * D:(tb + 1) * D],
                                    wp_t[c][:, nr * TCH:(nr + 1) * TCH],
                                    start=(c == 0), stop=False)
                        for nr in range(NPR):
                            nc.tensor.matmul(
                                pps[nr][:], ones_row[:],
                                bp_t[:, nr * TCH:(nr + 1) * TCH],
                                start=False, stop=True)
                        st = pstgp.tile([D, C], BF16, name="pst", tag="pst")
                        for nr in range(NPR):
                            nc.vector.tensor_copy(
                                st[:, nr * TCH:(nr + 1) * TCH], pps[nr][:])
                        nc.sync.dma_start(
                            part_dram[ic, tb * D:(tb + 1) * D, :], st[:])

                    nc.gpsimd.collective_compute(
                        "ReduceScatter",
                        mybir.AluOpType.add,
                        ins=[part_dram[ic, :, :]],
                        outs=[rs_out[ic, :, :]],
                        replica_groups=PAIRS,
                    )
                    nc.sync.dma_start(
                        out_ext[ic * HALF:(ic + 1) * HALF, :],
                        rs_out[ic, :, :])
    nc.finalize()
    return nc


def _prep_inputs(cfg: Cfg, x, w_attn, b_attn, w_proj, b_proj):
    """Host-side shard/cast. Returns in_maps (list of dicts per core)."""
    T, C, CP, HPC = cfg.T, cfg.C, cfg.CP, cfg.HPC
    bf = ml_dtypes.bfloat16
    wq = w_attn[:, 0:C]
    wk = w_attn[:, C:2 * C]
    wvf = w_attn[:, 2 * C:3 * C]
    bq, bk, bvf = b_attn[0:C], b_attn[C:2 * C], b_attn[2 * C:3 * C]

    masks = np.zeros((D, 4 * cfg.TCH), dtype=bf)
    f = np.arange(cfg.TCH)[None, :]
    p = np.arange(D)[:, None]
    for k in range(4):
        keep = (f - p >= 128 * k)
        masks[:, k * cfg.TCH:(k + 1) * cfg.TCH] = np.where(
            keep, 0.0, -30000.0).astype(bf)

    in_maps = []
    for core in range(N_CORES):
        b = core // 2
        g = core % 2
        h0 = g * HPC * D            # first col of this head group
        sl = slice(h0, h0 + CP)
        xTc = np.ascontiguousarray(x[b].T).astype(bf)
        wqk_cols = []
        for h in range(HPC):
            hs = slice(h0 + h * D, h0 + (h + 1) * D)
            wqk_cols.append(wq[:, hs])
            wqk_cols.append(wk[:, hs])
        wqk_c = np.concatenate(wqk_cols, axis=1).astype(bf)
        wv_c = wvf[:, sl].astype(bf)
        wp_c = w_proj[sl, :].astype(bf)
        bqk_cols = []
        for h in range(HPC):
            hs = slice(h0 + h * D, h0 + (h + 1) * D)
            bqk_cols.append(bq[hs])
            bqk_cols.append(bk[hs])
        bqk_c = np.ascontiguousarray(np.stack(bqk_cols, axis=1)).astype(np.float32)
        in_maps.append({
            "xT": xTc,
            "wqk": wqk_c,
            "wv": wv_c,
            "wp": wp_c,
            "bqk": bqk_c,
            "bv": bvf[sl].reshape(1, CP).astype(bf),
            "bp": (b_proj * (1.0 - g)).reshape(1, C).astype(bf),
            "masks": masks,
        })
    return in_maps


_CFG = Cfg()


def kernel(x, w_attn, b_attn, w_proj, b_proj, _trace=False, _cfg=None):
    from concourse.bass_utils import run_bass_kernel_spmd
    cfg = _cfg or _CFG
    x = np.asarray(x, dtype=np.float32)
    w_attn = np.asarray(w_attn, dtype=np.float32)
    b_attn = np.asarray(b_attn, dtype=np.float32)
    w_proj = np.asarray(w_proj, dtype=np.float32)
    b_proj = np.asarray(b_proj, dtype=np.float32)

    in_maps = _prep_inputs(cfg, x, w_attn, b_attn, w_proj, b_proj)
    nc = build_kernel(cfg)
    res = run_bass_kernel_spmd(nc, in_maps, list(range(N_CORES)), trace=_trace)
    HALF = cfg.TCH // 2
    outs = []
    for b in range(cfg.B):
        even = res.results[2 * b]["out"]     # [NIC*HALF, C] rows ic*512+[0,256)
        odd = res.results[2 * b + 1]["out"]  # rows ic*512+[256,512)
        full = np.empty((cfg.T, cfg.C), dtype=np.float32)
        NIC = cfg.T // cfg.TCH
        for ic in range(NIC):
            full[ic * cfg.TCH:ic * cfg.TCH + HALF] = \
                even[ic * HALF:(ic + 1) * HALF].astype(np.float32)
            full[ic * cfg.TCH + HALF:(ic + 1) * cfg.TCH] = \
                odd[ic * HALF:(ic + 1) * HALF].astype(np.float32)
        outs.append(full)
    full = np.stack(outs, axis=0)
    if _trace:
        kernel.last_exec_time_ns = res.exec_time_ns
        kernel.last_mean_exec_time_ns = res.mean_exec_time_ns
        kernel.last_scope_times = res.per_core_scope_times
    return full


# revision 17
# speedup vs baseline: 1.2156x; 1.0084x over previous
"""Trainium2 Bass kernel for causal self-attention (B=4, T=2048, C=2048, H=16).

Sharding: 8 cores = DP4 (batch) x TP2 (8 heads each). Per core:
  A1  q,k = (x @ Wqk)^T in [j, t] layout; k kept resident in SBUF,
      q staged to DRAM in 512-col chunks.
  A2  v = x @ Wv in [t, j] layout (stationary xT blocks), resident in SBUF.
  B   per 512-query chunk ic, per head: s_T = k'^T q' (psum groups of 2
      key-blocks), causal mask add, exp ([128,1024] ACT), av + ones-den
      matmuls, rec = ACT reciprocal, y = po * rec.
      Then proj partials for the chunk (own 8 heads, bf16) and a chunked
      pair ReduceScatter(add, bf16) overlapped with the next chunk.
Host: shard/cast/transpose inputs, assemble output, upcast to fp32.
"""
import math
import numpy as np
import ml_dtypes

import concourse.bass as bass
import concourse.bacc as bacc
import concourse.mybir as mybir
import concourse.tile as tile

F32 = mybir.dt.float32
BF16 = mybir.dt.bfloat16
AF = mybir.ActivationFunctionType

D = 128          # head dim (fixed: partition size)
N_CORES = 8
PAIRS = [[0, 1], [2, 3], [4, 5], [6, 7]]


class Cfg:
    def __init__(self, T=2048, H_TOT=16, HPC=8, B=4):
        self.T = T                    # sequence length
        self.H_TOT = H_TOT            # total heads
        self.HPC = HPC                # heads per core
        self.B = B
        self.C = H_TOT * D            # model dim
        self.CP = HPC * D             # per-core head cols
        self.TCH = 512                # query chunk width
        assert T % self.TCH == 0 and T % D == 0


def build_kernel(cfg: Cfg):
    T, C, CP, HPC, TCH = cfg.T, cfg.C, cfg.CP, cfg.HPC, cfg.TCH
    NC_CH = C // D                # c-chunks (contraction)
    NTB = T // D                  # t-blocks
    NIC = T // TCH                # query chunks
    NTR = T // 512                # t-ranges for A1 moving dim
    NPR = C // TCH                # proj n-ranges
    HALF = TCH // 2               # rows owned per core per chunk
    scale = 1.0 / math.sqrt(D)

    nc = bacc.Bacc()
    xT = nc.declare_dram_parameter("xT", [C, T], BF16, isOutput=False)
    wqk = nc.declare_dram_parameter("wqk", [C, 2 * CP], BF16, isOutput=False)
    wv = nc.declare_dram_parameter("wv", [C, CP], BF16, isOutput=False)
    wp = nc.declare_dram_parameter("wp", [CP, C], BF16, isOutput=False)
    bqk = nc.declare_dram_parameter("bqk", [D, 2 * HPC], F32, isOutput=False)
    bv = nc.declare_dram_parameter("bv", [1, CP], BF16, isOutput=False)
    bp = nc.declare_dram_parameter("bp", [1, C], BF16, isOutput=False)
    masks = nc.declare_dram_parameter("masks", [D, 4 * TCH], BF16, isOutput=False)
    out_ext = nc.declare_dram_parameter("out", [NIC * HALF, C], BF16, isOutput=True)

    q_dram = nc.dram_tensor("q_dram", [HPC, NIC, D, TCH], BF16)
    part_dram = nc.dram_tensor("part_dram", [NIC, TCH, C], BF16)
    rs_out = nc.dram_tensor("rs_out", [NIC, 2, HALF // 2, C], BF16)

    with tile.TileContext(nc) as tc:
        with (
            tc.tile_pool(name="const", bufs=1) as constp,
            tc.tile_pool(name="kres", bufs=1) as kres,
            tc.tile_pool(name="vres", bufs=1) as vres,
        ):
            bqk_t = constp.tile([D, 2 * HPC], F32, name="bqk_t")
            nc.sync.dma_start(bqk_t[:], bqk[:, :])
            bv_t = constp.tile([1, CP], BF16, name="bv_t")
            nc.sync.dma_start(bv_t[:], bv[:, :])
            ones_sq = constp.tile([D, D], BF16, name="ones_sq")
            nc.vector.memset(ones_sq[:], 1.0)
            ones_row = constp.tile([1, D], BF16, name="ones_row")
            nc.vector.memset(ones_row[:], 1.0)

            k_sb = [kres.tile([D, T], BF16, name=f"k{h}") for h in range(HPC)]
            v_sb = [vres.tile([D, CP], BF16, name=f"v{tb}") for tb in range(NTB)]

            # B-phase pools opened below xtp on the pool stack (LIFO close);
            # their tiles/DMAs are emitted after A1.
            wpp_ctx = tc.tile_pool(name="wpp", bufs=1)
            wpp = wpp_ctx.__enter__()
            qio_ctx = tc.tile_pool(name="qio", bufs=8)
            qio = qio_ctx.__enter__()

            # resident xT tiles, freed after A2
            xtp_ctx = tc.tile_pool(name="xtp", bufs=1)
            xtp = xtp_ctx.__enter__()
            xt = []
            for c in range(NC_CH):
                t = xtp.tile([D, T], BF16, name=f"xt{c}")
                nc.sync.dma_start(t[:], xT[c * D:(c + 1) * D, :])
                xt.append(t)

            # ---------------- A1: q (to DRAM) and k (resident) ----------
            with (
                tc.tile_pool(name="wqkp", bufs=2 * NC_CH) as wqkp,
                tc.tile_pool(name="pqk", bufs=8, space="PSUM") as pqkp,
                tc.tile_pool(name="qst", bufs=2) as qstp,
            ):
                for h in range(HPC):
                    for part in range(2):      # 0 = q, 1 = k
                        jb = 2 * h + part
                        wts = []
                        for c in range(NC_CH):
                            wt = wqkp.tile([D, D], BF16, name="wt", tag="wt")
                            nc.sync.dma_start(
                                wt[:], wqk[c * D:(c + 1) * D,
                                           jb * D:(jb + 1) * D])
                            wts.append(wt)
                        ps = [pqkp.tile([D, 512], F32, name="pq", tag="pq")
                              for _ in range(NTR)]
                        for c in range(NC_CH):
                            for tr in range(NTR):
                                nc.tensor.matmul(
                                    ps[tr][:], wts[c][:],
                                    xt[c][:, tr * 512:(tr + 1) * 512],
                                    start=(c == 0), stop=(c == NC_CH - 1))
                        if part == 1:
                            for tr in range(NTR):
                                nc.vector.tensor_scalar_add(
                                    k_sb[h][:, tr * 512:(tr + 1) * 512],
                                    ps[tr][:], bqk_t[:, jb:jb + 1])
                        else:
                            st = qstp.tile([D, T], BF16, name="qs", tag="qs")
                            for tr in range(NTR):
                                nc.vector.tensor_scalar_add(
                                    st[:, tr * 512:(tr + 1) * 512],
                                    ps[tr][:], bqk_t[:, jb:jb + 1])
                            for ic in range(NIC):
                                nc.sync.dma_start(
                                    q_dram[h, ic, :, :],
                                    st[:, ic * TCH:(ic + 1) * TCH])

            # wp + first-chunk q DMAs issued here so they overlap A2
            # compute instead of stalling B's first matmuls.
            wp_t = []
            for c in range(HPC):
                t = wpp.tile([D, C], BF16, name=f"wp{c}")
                nc.sync.dma_start(t[:], wp[c * D:(c + 1) * D, :])
                wp_t.append(t)
            qts0 = []
            for h in range(HPC):
                qt = qio.tile([D, TCH], BF16, name="qt", tag="qt")
                nc.sync.dma_start(qt[:], q_dram[h, 0, :, :])
                qts0.append(qt)

            # ---------------- A2: v (resident) ---------------------------
            with (
                tc.tile_pool(name="wvp", bufs=1) as wvp,
                tc.tile_pool(name="pv", bufs=4, space="PSUM") as pvp,
            ):
                wv_t = []
                for c in range(NC_CH):
                    t = wvp.tile([D, CP], BF16, name=f"wv{c}")
                    nc.sync.dma_start(t[:], wv[c * D:(c + 1) * D, :])
                    wv_t.append(t)
                NVR = CP // 512
                for tb in range(NTB):
                    ps = [pvp.tile([D, 512], F32, name="pv", tag="pv")
                          for _ in range(NVR)]
                    for c in range(NC_CH):
                        for vr in range(NVR):
                            nc.tensor.matmul(
                                ps[vr][:], xt[c][:, tb * D:(tb + 1) * D],
                                wv_t[c][:, vr * 512:(vr + 1) * 512],
                                start=(c == 0), stop=False)
                    for vr in range(NVR):
                        nc.tensor.matmul(
                            ps[vr][:], ones_row[:],
                            bv_t[:, vr * 512:(vr + 1) * 512],
                            start=False, stop=True)
                    for vr in range(NVR):
                        nc.vector.tensor_copy(
                            v_sb[tb][:, vr * 512:(vr + 1) * 512], ps[vr][:])
            xtp_ctx.__exit__(None, None, None)

            # ---------------- B: attention + chunked proj + RS -----------
            with (
                tc.tile_pool(name="const2", bufs=1) as const2p,
                tc.tile_pool(name="spool", bufs=2, space="PSUM") as spool,
                tc.tile_pool(name="acc", bufs=4, space="PSUM") as accp,
                tc.tile_pool(name="attp", bufs=3) as attp,
                tc.tile_pool(name="recp", bufs=2) as recp,
                tc.tile_pool(name="yres", bufs=18) as yresp,
                tc.tile_pool(name="pstg", bufs=3) as pstgp,
            ):
                bp_t = const2p.tile([1, C], BF16, name="bp_t")
                nc.sync.dma_start(bp_t[:], bp[:, :])
                mask_sb = const2p.tile([D, 4 * TCH], BF16, name="mask_sb")
                nc.sync.dma_start(mask_sb[:], masks[:, :])

                def p4_tb(pic, pys, tb):
                    """Proj partial for chunk pic, t-block tb (own heads)."""
                    pps = [accp.tile([D, TCH], F32, name="pp", tag="acc")
                           for _ in range(NPR)]
                    for c in range(HPC):
                        for nr in range(NPR):
                            nc.tensor.matmul(
                                pps[nr][:],
                                pys[c][:, tb * D:(tb + 1) * D],
                                wp_t[c][:, nr * TCH:(nr + 1) * TCH],
                                start=(c == 0), stop=False)
                    for nr in range(NPR):
                        nc.tensor.matmul(
                            pps[nr][:], ones_row[:],
                            bp_t[:, nr * TCH:(nr + 1) * TCH],
                            start=False, stop=True)
                    st = pstgp.tile([D, C], BF16, name="pst", tag="pst")
                    for nr in range(NPR):
                        nc.vector.tensor_copy(
                            st[:, nr * TCH:(nr + 1) * TCH], pps[nr][:])
                    nc.sync.dma_start(
                        part_dram[pic, tb * D:(tb + 1) * D, :], st[:])

                def rs_half(pic, hf):
                    """Pair ReduceScatter + output copy for half hf of pic."""
                    nc.gpsimd.collective_compute(
                        "ReduceScatter",
                        mybir.AluOpType.add,
                        ins=[part_dram[pic, hf * HALF:(hf + 1) * HALF, :]],
                        outs=[rs_out[pic, hf, :, :]],
                        replica_groups=PAIRS,
                    )
                    r0 = pic * HALF + hf * (HALF // 2)
                    nc.sync.dma_start(
                        out_ext[r0:r0 + HALF // 2, :], rs_out[pic, hf, :, :])

                def deferred(pic, pys, h):
                    """Interleave prev-chunk proj/RS after head h of cur."""
                    if pys is None:
                        return
                    if h % 2 == 1:
                        p4_tb(pic, pys, h // 2)
                    if h == 3:
                        rs_half(pic, 0)
                    elif h == 7:
                        rs_half(pic, 1)

                prev_ys = None
                for ic in range(NIC):
                    ntk = 4 * (ic + 1)
                    if ic == 0:
                        qts = qts0
                    else:
                        qts = []
                        for h in range(HPC):
                            qt = qio.tile([D, TCH], BF16, name="qt", tag="qt")
                            nc.sync.dma_start(qt[:], q_dram[h, ic, :, :])
                            qts.append(qt)
                    ys = []
                    for h in range(HPC):
                        po = accp.tile([D, TCH], F32, name="po", tag="acc")
                        pd = accp.tile([D, TCH], F32, name="pd", tag="acc")
                        for g in range(ntk // 2):
                            s = spool.tile([D, 2 * TCH], F32, name="sp",
                                           tag="sp")
                            for j in range(2):
                                tk = 2 * g + j
                                nc.tensor.matmul(
                                    s[:, j * TCH:(j + 1) * TCH],
                                    k_sb[h][:, tk * D:(tk + 1) * D],
                                    qts[h][:], start=True, stop=True)
                            for j in range(2):
                                tk = 2 * g + j
                                kdiag = tk - (ntk - 4)
                                if kdiag >= 0:
                                    nc.vector.tensor_add(
                                        s[:, j * TCH:(j + 1) * TCH],
                                        s[:, j * TCH:(j + 1) * TCH],
                                        mask_sb[:, kdiag * TCH:
                                                (kdiag + 1) * TCH])
                            att = attp.tile([D, 2 * TCH], BF16, name="att",
                                            tag="att")
                            nc.scalar.activation(att[:], s[:], AF.Exp,
                                                 bias=0.0, scale=scale)
                            for j in range(2):
                                tk = 2 * g + j
                                nc.tensor.matmul(
                                    po[:], v_sb[tk][:, h * D:(h + 1) * D],
                                    att[:, j * TCH:(j + 1) * TCH],
                                    start=(tk == 0), stop=(tk == ntk - 1))
                            for j in range(2):
                                tk = 2 * g + j
                                nc.tensor.matmul(
                                    pd[:], ones_sq[:],
                                    att[:, j * TCH:(j + 1) * TCH],
                                    start=(tk == 0), stop=(tk == ntk - 1))
                        rec = recp.tile([D, TCH], F32, name="rec", tag="rec")
                        nc.vector.reciprocal_approx_fast(rec[:], pd[:])
                        yt = yresp.tile([D, TCH], BF16, name="yt", tag="yt")
                        nc.vector.tensor_mul(yt[:], po[:], rec[:])
                        ys.append(yt)
                        deferred(ic - 1, prev_ys, h)
                    prev_ys = ys

                # drain: proj + RS for the final chunk
                for tb in range(NIC):
                    p4_tb(NIC - 1, prev_ys, tb)
                    if tb == 1:
                        rs_half(NIC - 1, 0)
                rs_half(NIC - 1, 1)
            qio_ctx.__exit__(None, None, None)
            wpp_ctx.__exit__(None, None, None)
    nc.finalize()
    return nc


def _prep_inputs(cfg: Cfg, x, w_attn, b_attn, w_proj, b_proj):
    """Host-side shard/cast. Returns in_maps (list of dicts per core)."""
    T, C, CP, HPC = cfg.T, cfg.C, cfg.CP, cfg.HPC
    bf = ml_dtypes.bfloat16
    wq = w_attn[:, 0:C]
    wk = w_attn[:, C:2 * C]
    wvf = w_attn[:, 2 * C:3 * C]
    bq, bk, bvf = b_attn[0:C], b_attn[C:2 * C], b_attn[2 * C:3 * C]

    masks = np.zeros((D, 4 * cfg.TCH), dtype=bf)
    f = np.arange(cfg.TCH)[None, :]
    p = np.arange(D)[:, None]
    for k in range(4):
        keep = (f - p >= 128 * k)
        masks[:, k * cfg.TCH:(k + 1) * cfg.TCH] = np.where(
            keep, 0.0, -30000.0).astype(bf)

    in_maps = []
    for core in range(N_CORES):
        b = core // 2
        g = core % 2
        h0 = g * HPC * D            # first col of this head group
        sl = slice(h0, h0 + CP)
        xTc = np.ascontiguousarray(x[b].T).astype(bf)
        wqk_cols = []
        for h in range(HPC):
            hs = slice(h0 + h * D, h0 + (h + 1) * D)
            wqk_cols.append(wq[:, hs])
            wqk_cols.append(wk[:, hs])
        wqk_c = np.concatenate(wqk_cols, axis=1).astype(bf)
        wv_c = wvf[:, sl].astype(bf)
        wp_c = w_proj[sl, :].astype(bf)
        bqk_cols = []
        for h in range(HPC):
            hs = slice(h0 + h * D, h0 + (h + 1) * D)
            bqk_cols.append(bq[hs])
            bqk_cols.append(bk[hs])
        bqk_c = np.ascontiguousarray(np.stack(bqk_cols, axis=1)).astype(np.float32)
        in_maps.append({
            "xT": xTc,
            "wqk": wqk_c,
            "wv": wv_c,
            "wp": wp_c,
            "bqk": bqk_c,
            "bv": bvf[sl].reshape(1, CP).astype(bf),
            "bp": (b_proj * (1.0 - g)).reshape(1, C).astype(bf),
            "masks": masks,
        })
    return in_maps


_CFG = Cfg()


def kernel(x, w_attn, b_attn, w_proj, b_proj, _trace=False, _cfg=None):
    from concourse.bass_utils import run_bass_kernel_spmd
    cfg = _cfg or _CFG
    x = np.asarray(x, dtype=np.float32)
    w_attn = np.asarray(w_attn, dtype=np.float32)
    b_attn = np.asarray(b_attn, dtype=np.float32)
    w_proj = np.asarray(w_proj, dtype=np.float32)
    b_proj = np.asarray(b_proj, dtype=np.float32)

    in_maps = _prep_inputs(cfg, x, w_attn, b_attn, w_proj, b_proj)
    nc = build_kernel(cfg)
    res = run_bass_kernel_spmd(nc, in_maps, list(range(N_CORES)), trace=_trace)
    HALF = cfg.TCH // 2
    Q = HALF // 2
    NIC = cfg.T // cfg.TCH
    outs = []
    for b in range(cfg.B):
        even = res.results[2 * b]["out"]     # rows ic*512+hf*256+[0,128)
        odd = res.results[2 * b + 1]["out"]  # rows ic*512+hf*256+[128,256)
        full = np.empty((cfg.T, cfg.C), dtype=np.float32)
        for ic in range(NIC):
            for hf in range(2):
                src = slice(ic * HALF + hf * Q, ic * HALF + (hf + 1) * Q)
                dst = ic * cfg.TCH + hf * HALF
                full[dst:dst + Q] = even[src].astype(np.float32)
                full[dst + Q:dst + 2 * Q] = odd[src].astype(np.float32)
        outs.append(full)
    full = np.stack(outs, axis=0)
    if _trace:
        kernel.last_exec_time_ns = res.exec_time_ns
        kernel.last_mean_exec_time_ns = res.mean_exec_time_ns
        kernel.last_scope_times = res.per_core_scope_times
    return full
